# revision 28
# baseline (speedup 1.0000x reference)
"""Bayesian triplet loss on 8 Trainium2 NeuronCores (Bass/Tile, SPMD).

Reference semantics:
  u   = clip(uncertainties, 1e-6, 1.0)
  d2[i,j] = ||e_i - e_j||^2            (Gram trick: n_i + n_j - 2 e_i.e_j)
  S[i,j]  = sum_k (e_ik - e_jk)^2 u_ik^2 = a_i - 2*M1[i,j] + M2[i,j]
            (a_i = sum u2*e^2, M1 = (u2*e)E^T, M2 = u2 (E*E)^T)
  mining: hardest positive (max d2 same-label), hardest negative
          (min d2 diff-label).
  per_triplet = softplus(10*(d_pos - d_neg + 0.3*(1+sigma)))/10,
          sigma = sqrt(S_pos/d2_pos + S_neg/d2_neg + 3e-8)
  loss = sum(valid*per_triplet)/max(sum(valid),1) + 0.05*mean(u)

Implementation notes:
  * Inputs are shipped host-transposed (contraction dim d on partitions)
    so every DMA lands with contiguous rows and the TensorEngine needs
    no on-device transposes.  E^T is also packed to bf16 on the host --
    identical values to the on-device cast it replaces, at half the DMA
    bytes.  All arithmetic (distances, masks, mining, loss) runs
    on-device; the host only does layout prep and the final 8-way
    partial-sum combine.
  * A short burst of dummy matmuls on constant data runs while the
    input DMAs land so PE_HAM releases the 1.2 GHz cold-clock throttle
    before the real matmuls issue.
  * Label masks are folded into the pairwise PSUM via a one-hot matmul:
    PSUM_A = d2 + V*same(i,j), V=65536 (all d2 < 500 here).  One
    free-dim max mines the hardest positive (same-label entries
    dominate), one min mines the hardest negative.  d2_pos is recovered
    exactly as max - V (both live in the 2^16 binade).
  * The diagonal needs no explicit mask: d2_ii ~ 0 can never be the
    same-label max, and singleton-label anchors have ~0 probability.
  * S at the argmax/argmin is recovered by exact float equality against
    the PSUM values, multiplied by S and accumulated on the Scalar
    engine -- no argmax/gather instruction needed.
  * n_i, n_j and a_i are all injected into PSUM with all-ones matmuls
    (one extra pass each), so no partition-direction reductions exist
    anywhere except the final [128,4] -> [1,4] ones-matmul.
  * Sharding: anchors (batch rows) split 8 ways; embeddings replicated
    per core, so no collectives.  Each core emits [1,4] partial sums;
    the host combines them (the usual data-parallel loss gather).
"""

import sys

if "/opt/trn_rl_repo" not in sys.path:
    sys.path.insert(0, "/opt/trn_rl_repo")

import numpy as np

import concourse.bacc as bacc
import concourse.mybir as mybir
from concourse import tile
from concourse.bass_utils import run_bass_kernel_spmd

# Force every activation into the one table that contains all functions
# this kernel uses (ln, exp, abs, relu, square, copy, identity).  The
# default first-match placement alternates natural_log <-> exp_and_others
# tables, costing a 1.3us ACT_TABLE_LOAD per transition.  Set ids must
# keep their act_info.json positions, so empty the other sets instead of
# reordering.
_ORIG_GAT = bacc.get_activation_tables


def _gat_single_set(arch):
    tabs = _ORIG_GAT(arch)
    keep = "natural_log_exp_and_others"
    if keep in tabs:
        return {n: (f if n == keep else set()) for n, f in tabs.items()}
    return tabs


bacc.get_activation_tables = _gat_single_set

B, D = 1024, 128
NUM_CLASSES = 64
N_CORES = 8
SH = B // N_CORES  # 128 anchor rows per core
JT = 2             # two 512-wide column tiles
JW = B // JT

F32 = mybir.dt.float32
BF16 = mybir.dt.bfloat16
NP_BF16 = mybir.dt.np(BF16)

SAME_V = 65536.0   # same-label offset; exact in bf16/f32
ALU = mybir.AluOpType
AF = mybir.ActivationFunctionType


def build_nc():
    nc = bacc.Bacc("TRN2", target_bir_lowering=False, debug=False,
                   num_devices=N_CORES)

    etb_in = nc.dram_tensor("etb", [D, B], BF16, kind="ExternalInput")
    aux = nc.dram_tensor("aux", [D, 2 * SH], F32, kind="ExternalInput")
    ohx = nc.dram_tensor("ohx", [NUM_CLASSES, SH + B], BF16,
                         kind="ExternalInput")
    out = nc.dram_tensor("out", [1, 4], F32, kind="ExternalOutput")

    with tile.TileContext(nc) as tc:
        with (
            tc.tile_pool(name="singles", bufs=1) as singles,
            tc.tile_pool(name="work", bufs=1) as work,
            tc.tile_pool(name="pmain", bufs=1, space="PSUM") as pmain,
        ):
            # ---------------- loads first (3 DMAs from 3 engines so the
            # trigger/SWDGE prep overlaps; everything else queues behind) --
            # ones_b first: it gates the PE warm-up stream
            ones_b = singles.tile([128, JW], BF16)
            nc.vector.memset(ones_b[:], 1.0)

            aux_sb = work.tile([D, 2 * SH], F32)
            nc.sync.dma_start(aux_sb[:], aux[:, :])
            etanc_sb = aux_sb[:, 0:SH]
            utanc_sb = aux_sb[:, SH:2 * SH]
            etb = work.tile([D, B], BF16)        # E^T bf16
            nc.sync.dma_start(etb[:, :JW], etb_in[:, :JW])
            nc.sync.dma_start(etb[:, JW:], etb_in[:, JW:])
            ohx_sb = work.tile([NUM_CLASSES, SH + B], BF16)
            nc.gpsimd.dma_start(ohx_sb[:], ohx[:, :])
            oha = ohx_sb[:, 0:SH]
            ohl = ohx_sb[:, SH:SH + B]

            # ---------------- remaining constants -----------------
            ones_col = singles.tile([128, 1], F32)
            nc.gpsimd.memset(ones_col[:], 1.0)
            b_sig = singles.tile([128, 1], F32)
            nc.gpsimd.memset(b_sig[:], 3.0e-8)
            b_three = singles.tile([128, 1], F32)
            nc.gpsimd.memset(b_three[:], 3.0)

            # ---------------- stats tile (written piecemeal) ------------
            stats = singles.tile([128, 4], F32)

            # PE warm-up: ~3.5us of dummy matmuls on constant data while
            # the input DMAs land, so PE_HAM releases the clock throttle
            # (1.2 -> 2.4 GHz) before the real matmuls issue.
            with tc.tile_pool(name="pwarm", bufs=1, space="PSUM") as pwarm:
                psW = pwarm.tile([128, JW], F32)
                for _ in range(9):
                    nc.tensor.matmul(psW[:], ones_b[:, :128], ones_b[:],
                                     start=True, stop=True)

            # ---------------- prep: anchor chain first (critical path) ---
            emtb = work.tile([D, SH], BF16)      # anchor E^T bf16
            nc.vector.tensor_copy(emtb[:], etanc_sb)
            nemtb = work.tile([D, SH], BF16)     # -2 * anchor E^T
            nc.scalar.mul(nemtb[:], emtb[:], -2.0)
            eetmb = work.tile([D, SH], BF16)     # anchor (E^T)^2
            nc.scalar.square(eetmb[:], emtb[:])

            # u clip (+ total-sum for the regularizer), square
            ucl = work.tile([D, SH], F32)
            nc.vector.tensor_scalar(ucl[:], utanc_sb, 1.0e-6, 1.0,
                                    op0=ALU.max, op1=ALU.min)
            u2t32 = work.tile([D, SH], F32)
            nc.scalar.square(u2t32[:], ucl[:])
            u2tb = work.tile([D, SH], BF16)
            nc.vector.tensor_copy(u2tb[:], u2t32[:])
            w2b = work.tile([D, SH], BF16)       # -2 * u2^T * E^T
            nc.vector.scalar_tensor_tensor(w2b[:], u2tb[:], -2.0, emtb[:],
                                           op0=ALU.mult, op1=ALU.mult)
            w2ee = work.tile([D, SH], BF16)      # u2^T * (E^T)^2  (a_i lhsT)
            nc.vector.tensor_tensor(w2ee[:], u2tb[:], eetmb[:], op=ALU.mult)
            nc.vector.tensor_reduce(stats[:, 2:3], ucl[:],
                                    axis=mybir.AxisListType.X, op=ALU.add)

            # bulk E^T squares
            eetb = work.tile([D, B], BF16)       # (E^T)^2
            nc.scalar.square(eetb[:, :JW], etb[:, :JW])
            nc.scalar.square(eetb[:, JW:], etb[:, JW:])

            # ---------------- main matmuls -----------------
            # PSUM_A = d2 + V*same = -2G + n_i + n_j + V*same01
            # PSUM_B = S = M2 - 2*M1 + a_i
            psA = pmain.tile([128, JT, JW], F32)
            psB = pmain.tile([128, JT, JW], F32)
            s_sb = work.tile([128, B], BF16)
            pr2 = singles.tile([128, 2], F32)    # per-half max of psA
            nv2 = singles.tile([128, 2], F32)    # per-half min of psA
            shalf0 = singles.tile([128, 2], F32)  # half0 [S_pos, S_neg]
            shalf1 = singles.tile([128, 2], F32)  # half1 [S_pos, S_neg]
            for jt in range(JT):
                sl = slice(jt * JW, (jt + 1) * JW)
                nc.tensor.matmul(psA[:, jt, :], oha, ohl[:, sl.start:sl.stop],
                                 start=True, stop=False)          # + V*same
                nc.tensor.matmul(psA[:, jt, :], nemtb[:], etb[:, sl],
                                 start=False, stop=False)         # -2 G
                nc.tensor.matmul(psA[:, jt, :], eetmb[:], ones_b[:, :JW],
                                 start=False, stop=False)         # + n_i
                nc.tensor.matmul(psA[:, jt, :], ones_b[:, :128], eetb[:, sl],
                                 start=False, stop=True)          # + n_j
                # mining as soon as this psA half is complete
                nc.vector.tensor_reduce(pr2[:, jt:jt + 1], psA[:, jt, :],
                                        axis=mybir.AxisListType.X, op=ALU.max)
                nc.vector.tensor_reduce(nv2[:, jt:jt + 1], psA[:, jt, :],
                                        axis=mybir.AxisListType.X, op=ALU.min)
            for jt in range(JT):
                sl = slice(jt * JW, (jt + 1) * JW)
                nc.tensor.matmul(psB[:, jt, :], w2b[:], etb[:, sl],
                                 start=True, stop=False)          # -2 M1
                nc.tensor.matmul(psB[:, jt, :], u2tb[:], eetb[:, sl],
                                 start=False, stop=False)         # M2
                nc.tensor.matmul(psB[:, jt, :], w2ee[:], ones_b[:, :JW],
                                 start=False, stop=True)          # + a_i
                nc.scalar.activation(s_sb[:, sl], psB[:, jt, :], AF.Relu)
                mp = work.tile([128, JW], BF16, tag="mp")
                nc.vector.scalar_tensor_tensor(mp[:], psA[:, jt, :],
                                               pr2[:, jt:jt + 1],
                                               s_sb[:, sl],
                                               op0=ALU.is_equal, op1=ALU.mult)
                scr_p = work.tile([128, JW], BF16, tag="scr_p")
                sh = shalf0 if jt == 0 else shalf1
                nc.scalar.activation(scr_p[:], mp[:], AF.Copy,
                                     accum_out=sh[:, 0:1])
                mn = work.tile([128, JW], BF16, tag="mn")
                nc.vector.scalar_tensor_tensor(mn[:], psA[:, jt, :],
                                               nv2[:, jt:jt + 1],
                                               s_sb[:, sl],
                                               op0=ALU.is_equal, op1=ALU.mult)
                scr_n = work.tile([128, JW], BF16, tag="scr_n")
                nc.scalar.activation(scr_n[:], mn[:], AF.Copy,
                                     accum_out=sh[:, 1:2])

            # ---------------- mining merge --------------------
            pos_raw = singles.tile([128, 1], F32)   # V + d2_pos
            nc.vector.tensor_reduce(pos_raw[:], pr2[:],
                                    axis=mybir.AxisListType.X, op=ALU.max)
            pv = singles.tile([128, 2], F32)        # [:,0]=d2_pos [:,1]=d2_neg
            nc.vector.tensor_reduce(pv[:, 1:2], nv2[:],
                                    axis=mybir.AxisListType.X, op=ALU.min)
            nc.vector.tensor_scalar(pv[:, 0:1], pos_raw[:], -SAME_V, None,
                                    op0=ALU.add)

            s_sel = singles.tile([128, 2], F32)     # S at argmax / argmin
            wsel = work.tile([128, 2], F32)
            nc.vector.tensor_tensor(wsel[:, 0:1], pr2[:, 0:1], pr2[:, 1:2],
                                    op=ALU.is_ge)   # half0 holds global max?
            nc.vector.tensor_tensor(wsel[:, 1:2], nv2[:, 0:1], nv2[:, 1:2],
                                    op=ALU.is_le)   # half0 holds global min?
            dsel = work.tile([128, 2], F32)
            nc.vector.tensor_tensor(dsel[:], shalf0[:], shalf1[:],
                                    op=ALU.subtract)
            nc.vector.tensor_tensor(dsel[:], dsel[:], wsel[:], op=ALU.mult)
            nc.vector.tensor_tensor(s_sel[:], dsel[:], shalf1[:], op=ALU.add)

            # ---------------- per-anchor tail ([128,*] small ops) --------
            t_pool = work
            # valid = hardest negative exists (d2_neg < 1e4)
            nc.vector.tensor_scalar(stats[:, 1:2], pv[:, 1:2], 1.0e4, None,
                                    op0=ALU.is_lt)
            # triple = [u2sum, 100*d2_pos, 100*d2_neg]; then one packed
            # exp(0.5*ln(. + 3e-8)) gives [sigma, 10*d_pos, 10*d_neg]
            # (the +3e-8 bias is microscopic vs 100*d2 ~ 1e4).
            pq = t_pool.tile([128, 2], F32)         # guarded d2 (for recip)
            nc.vector.tensor_scalar(pq[:], pv[:], 1.0e-6, None, op0=ALU.max)
            triple = t_pool.tile([128, 3], F32)
            nc.vector.tensor_scalar(triple[:, 1:3], pv[:], 1.0e-6, 100.0,
                                    op0=ALU.max, op1=ALU.mult)
            rcp = t_pool.tile([128, 2], F32)
            nc.vector.reciprocal(rcp[:], pq[:])
            u2_pn = t_pool.tile([128, 2], F32)
            nc.vector.tensor_tensor(u2_pn[:], s_sel[:], rcp[:], op=ALU.mult)
            nc.vector.tensor_reduce(triple[:, 0:1], u2_pn[:],
                                    axis=mybir.AxisListType.X, op=ALU.add)
            lt3 = t_pool.tile([128, 3], F32)
            nc.scalar.activation(lt3[:], triple[:], AF.Ln,
                                 bias=b_sig[:], scale=1.0)
            e3 = t_pool.tile([128, 3], F32)         # sigma, 10*d_pos, 10*d_neg
            nc.scalar.activation(e3[:], lt3[:], AF.Exp, scale=0.5)
            sig = e3[:, 0:1]
            pre = t_pool.tile([128, 1], F32)        # 10*(d_pos - d_neg)
            nc.vector.tensor_tensor(pre[:], e3[:, 1:2], e3[:, 2:3],
                                    op=ALU.subtract)
            raw = t_pool.tile([128, 1], F32)        # pre + 3*sigma
            nc.vector.scalar_tensor_tensor(raw[:], sig, 3.0, pre[:],
                                           op0=ALU.mult, op1=ALU.add)
            # softplus(x) = relu(x) + ln(1 + exp(-|x|)), x = raw + 3
            ax = t_pool.tile([128, 1], F32)
            nc.scalar.activation(ax[:], raw[:], AF.Abs,
                                 bias=b_three[:], scale=1.0)
            en = t_pool.tile([128, 1], F32)
            nc.scalar.activation(en[:], ax[:], AF.Exp, scale=-1.0)
            l1p = t_pool.tile([128, 1], F32)
            nc.scalar.activation(l1p[:], en[:], AF.Ln, bias=1.0, scale=1.0)
            rl = t_pool.tile([128, 1], F32)
            nc.vector.tensor_scalar(rl[:], raw[:], 3.0, 0.0,
                                    op0=ALU.add, op1=ALU.max)
            pt10 = t_pool.tile([128, 1], F32)       # softplus(10*raw_ref)
            nc.vector.tensor_tensor(pt10[:], rl[:], l1p[:], op=ALU.add)
            nc.vector.tensor_tensor(stats[:, 0:1], pt10[:], stats[:, 1:2],
                                    op=ALU.mult)
            nc.gpsimd.memset(stats[:, 3:4], 0.0)

            # ---------------- final partition reduction -----------------
            ps_out = pmain.tile([1, 4], F32)
            nc.tensor.matmul(ps_out[:], ones_col[:], stats[:],
                             start=True, stop=True)
            out_sb = singles.tile([1, 4], F32)
            nc.vector.tensor_copy(out_sb[:], ps_out[:])
            nc.sync.dma_start(out[:, :], out_sb[:])

    nc.compile()
    return nc


_NC = None


def _get_nc():
    global _NC
    if _NC is None:
        _NC = build_nc()
    return _NC


def build_in_maps(embeddings, uncertainties, labels):
    emb = np.asarray(embeddings, dtype=np.float32)
    unc = np.asarray(uncertainties, dtype=np.float32)
    lab = np.asarray(labels).reshape(B).astype(np.int64)
    etf = np.ascontiguousarray(emb.T)                  # [D, B]
    etb16 = np.ascontiguousarray(etf.astype(NP_BF16))  # bf16 E^T for PE
    utf = np.ascontiguousarray(unc.T)                  # [D, B]
    onehot = np.zeros((NUM_CLASSES, B), np.float32)
    onehot[lab, np.arange(B)] = 1.0
    ohall = np.ascontiguousarray(onehot.astype(NP_BF16))
    ohv = (SAME_V * onehot).astype(NP_BF16)
    in_maps = []
    for c in range(N_CORES):
        r0, r1 = c * SH, (c + 1) * SH
        in_maps.append({
            "etb": etb16,
            "aux": np.ascontiguousarray(
                np.concatenate([etf[:, r0:r1], utf[:, r0:r1]], axis=1)),
            "ohx": np.ascontiguousarray(
                np.concatenate([ohv[:, r0:r1], ohall], axis=1)),
        })
    return in_maps


def finalize(results):
    stats = np.stack([np.asarray(results[c]["out"]).reshape(4)
                      for c in range(N_CORES)])
    tot = stats.sum(axis=0)
    main = (tot[0] / 10.0) / max(tot[1], 1.0)
    reg = tot[2] / (B * D)
    return np.float32(main + 0.05 * reg)


def kernel(embeddings, uncertainties, labels):
    nc = _get_nc()
    in_maps = build_in_maps(embeddings, uncertainties, labels)
    res = run_bass_kernel_spmd(nc, in_maps, core_ids=list(range(N_CORES)))
    return finalize(res.results)


# revision 29
# speedup vs baseline: 1.0086x; 1.0086x over previous
"""Bayesian triplet loss on 8 Trainium2 NeuronCores (Bass/Tile, SPMD).

Reference semantics:
  u   = clip(uncertainties, 1e-6, 1.0)
  d2[i,j] = ||e_i - e_j||^2            (Gram trick: n_i + n_j - 2 e_i.e_j)
  S[i,j]  = sum_k (e_ik - e_jk)^2 u_ik^2 = a_i - 2*M1[i,j] + M2[i,j]
            (a_i = sum u2*e^2, M1 = (u2*e)E^T, M2 = u2 (E*E)^T)
  mining: hardest positive (max d2 same-label), hardest negative
          (min d2 diff-label).
  per_triplet = softplus(10*(d_pos - d_neg + 0.3*(1+sigma)))/10,
          sigma = sqrt(S_pos/d2_pos + S_neg/d2_neg + 3e-8)
  loss = sum(valid*per_triplet)/max(sum(valid),1) + 0.05*mean(u)

Implementation notes:
  * Inputs are shipped host-transposed (contraction dim d on partitions)
    so every DMA lands with contiguous rows and the TensorEngine needs
    no on-device transposes.  E^T is also packed to bf16 on the host --
    identical values to the on-device cast it replaces, at half the DMA
    bytes.  All arithmetic (distances, masks, mining, loss) runs
    on-device; the host only does layout prep and the final 8-way
    partial-sum combine.
  * A short burst of dummy matmuls on constant data runs while the
    input DMAs land so PE_HAM releases the 1.2 GHz cold-clock throttle
    before the real matmuls issue.
  * Label masks are folded into the pairwise PSUM via a one-hot matmul:
    PSUM_A = d2 + V*same(i,j), V=65536 (all d2 < 500 here).  One
    free-dim max mines the hardest positive (same-label entries
    dominate), one min mines the hardest negative.  d2_pos is recovered
    exactly as max - V (both live in the 2^16 binade).
  * The diagonal needs no explicit mask: d2_ii ~ 0 can never be the
    same-label max, and singleton-label anchors have ~0 probability.
  * S at the argmax/argmin is recovered by exact float equality against
    the PSUM values, multiplied by S and accumulated on the Scalar
    engine -- no argmax/gather instruction needed.
  * n_i, n_j and a_i are all injected into PSUM with all-ones matmuls
    (one extra pass each), so no partition-direction reductions exist
    anywhere except the final [128,4] -> [1,4] ones-matmul.
  * Sharding: anchors (batch rows) split 8 ways; embeddings replicated
    per core, so no collectives.  Each core emits [1,4] partial sums;
    the host combines them (the usual data-parallel loss gather).
"""

import sys

if "/opt/trn_rl_repo" not in sys.path:
    sys.path.insert(0, "/opt/trn_rl_repo")

import numpy as np

import concourse.bacc as bacc
import concourse.mybir as mybir
from concourse import tile
from concourse.bass_utils import run_bass_kernel_spmd

# Force every activation into the one table that contains all functions
# this kernel uses (ln, exp, abs, relu, square, copy, identity).  The
# default first-match placement alternates natural_log <-> exp_and_others
# tables, costing a 1.3us ACT_TABLE_LOAD per transition.  Set ids must
# keep their act_info.json positions, so empty the other sets instead of
# reordering.
_ORIG_GAT = bacc.get_activation_tables


def _gat_single_set(arch):
    tabs = _ORIG_GAT(arch)
    keep = "natural_log_exp_and_others"
    if keep in tabs:
        return {n: (f if n == keep else set()) for n, f in tabs.items()}
    return tabs


bacc.get_activation_tables = _gat_single_set

B, D = 1024, 128
NUM_CLASSES = 64
N_CORES = 8
SH = B // N_CORES  # 128 anchor rows per core
JT = 2             # two 512-wide column tiles
JW = B // JT

F32 = mybir.dt.float32
BF16 = mybir.dt.bfloat16
NP_BF16 = mybir.dt.np(BF16)

SAME_V = 65536.0   # same-label offset; exact in bf16/f32
ALU = mybir.AluOpType
AF = mybir.ActivationFunctionType


def build_nc():
    nc = bacc.Bacc("TRN2", target_bir_lowering=False, debug=False,
                   num_devices=N_CORES)

    etb_in = nc.dram_tensor("etb", [D, B], BF16, kind="ExternalInput")
    aux = nc.dram_tensor("aux", [D, 2 * SH], F32, kind="ExternalInput")
    ohx = nc.dram_tensor("ohx", [NUM_CLASSES, SH + B], BF16,
                         kind="ExternalInput")
    out = nc.dram_tensor("out", [1, 4], F32, kind="ExternalOutput")

    with tile.TileContext(nc) as tc:
        with (
            tc.tile_pool(name="singles", bufs=1) as singles,
            tc.tile_pool(name="work", bufs=1) as work,
            tc.tile_pool(name="pmain", bufs=1, space="PSUM") as pmain,
        ):
            # ---------------- loads first (3 DMAs from 3 engines so the
            # trigger/SWDGE prep overlaps; everything else queues behind) --
            # ones_b first: it gates the PE warm-up stream
            ones_b = singles.tile([128, JW], BF16)
            nc.vector.memset(ones_b[:], 1.0)

            aux_sb = work.tile([D, 2 * SH], F32)
            nc.sync.dma_start(aux_sb[:], aux[:, :])
            etanc_sb = aux_sb[:, 0:SH]
            utanc_sb = aux_sb[:, SH:2 * SH]
            etb = work.tile([D, B], BF16)        # E^T bf16
            nc.sync.dma_start(etb[:, :JW], etb_in[:, :JW])
            nc.sync.dma_start(etb[:, JW:], etb_in[:, JW:])
            ohx_sb = work.tile([NUM_CLASSES, SH + B], BF16)
            nc.gpsimd.dma_start(ohx_sb[:], ohx[:, :])
            oha = ohx_sb[:, 0:SH]
            ohl = ohx_sb[:, SH:SH + B]

            # ---------------- remaining constants -----------------
            ones_col = singles.tile([128, 1], F32)
            nc.gpsimd.memset(ones_col[:], 1.0)
            b_sig = singles.tile([128, 1], F32)
            nc.gpsimd.memset(b_sig[:], 3.0e-8)
            b_three = singles.tile([128, 1], F32)
            nc.gpsimd.memset(b_three[:], 3.0)

            # ---------------- stats tile (written piecemeal) ------------
            stats = singles.tile([128, 4], F32)

            # PE warm-up: ~3.5us of dummy matmuls on constant data while
            # the input DMAs land, so PE_HAM releases the clock throttle
            # (1.2 -> 2.4 GHz) before the real matmuls issue.
            with tc.tile_pool(name="pwarm", bufs=1, space="PSUM") as pwarm:
                psW = pwarm.tile([128, JW], F32)
                for _ in range(9):
                    nc.tensor.matmul(psW[:], ones_b[:, :128], ones_b[:],
                                     start=True, stop=True)

            # ---------------- prep: anchor chain first (critical path) ---
            emtb = work.tile([D, SH], BF16)      # anchor E^T bf16
            nc.vector.tensor_copy(emtb[:], etanc_sb)
            nemtb = work.tile([D, SH], BF16)     # -2 * anchor E^T
            nc.scalar.mul(nemtb[:], emtb[:], -2.0)
            eetmb = work.tile([D, SH], BF16)     # anchor (E^T)^2
            nc.scalar.square(eetmb[:], emtb[:])

            # u clip (+ total-sum for the regularizer), square
            ucl = work.tile([D, SH], F32)
            nc.vector.tensor_scalar(ucl[:], utanc_sb, 1.0e-6, 1.0,
                                    op0=ALU.max, op1=ALU.min)
            u2t32 = work.tile([D, SH], F32)
            nc.scalar.square(u2t32[:], ucl[:])
            u2tb = work.tile([D, SH], BF16)
            nc.vector.tensor_copy(u2tb[:], u2t32[:])
            w2b = work.tile([D, SH], BF16)       # -2 * u2^T * E^T
            nc.vector.scalar_tensor_tensor(w2b[:], u2tb[:], -2.0, emtb[:],
                                           op0=ALU.mult, op1=ALU.mult)
            w2ee = work.tile([D, SH], BF16)      # u2^T * (E^T)^2  (a_i lhsT)
            nc.vector.tensor_tensor(w2ee[:], u2tb[:], eetmb[:], op=ALU.mult)
            nc.vector.tensor_reduce(stats[:, 2:3], ucl[:],
                                    axis=mybir.AxisListType.X, op=ALU.add)

            # bulk E^T squares
            eetb = work.tile([D, B], BF16)       # (E^T)^2
            nc.scalar.square(eetb[:, :JW], etb[:, :JW])
            nc.scalar.square(eetb[:, JW:], etb[:, JW:])

            # ---------------- main matmuls -----------------
            # PSUM_A = d2 + V*same = -2G + n_i + n_j + V*same01
            # PSUM_B = S = M2 - 2*M1 + a_i
            psA = pmain.tile([128, JT, JW], F32)
            psB = pmain.tile([128, JT, JW], F32)
            s_sb = work.tile([128, B], BF16)
            pr2 = singles.tile([128, 2], F32)    # per-half max of psA
            nv2 = singles.tile([128, 2], F32)    # per-half min of psA
            shalf0 = singles.tile([128, 2], F32)  # half0 [S_pos, S_neg]
            shalf1 = singles.tile([128, 2], F32)  # half1 [S_pos, S_neg]
            for jt in range(JT):
                sl = slice(jt * JW, (jt + 1) * JW)
                nc.tensor.matmul(psA[:, jt, :], oha, ohl[:, sl.start:sl.stop],
                                 start=True, stop=False)          # + V*same
                nc.tensor.matmul(psA[:, jt, :], nemtb[:], etb[:, sl],
                                 start=False, stop=False)         # -2 G
                nc.tensor.matmul(psA[:, jt, :], eetmb[:], ones_b[:, :JW],
                                 start=False, stop=False)         # + n_i
                nc.tensor.matmul(psA[:, jt, :], ones_b[:, :128], eetb[:, sl],
                                 start=False, stop=True)          # + n_j
                # mining as soon as this psA half is complete
                nc.vector.tensor_reduce(pr2[:, jt:jt + 1], psA[:, jt, :],
                                        axis=mybir.AxisListType.X, op=ALU.max)
                nc.vector.tensor_reduce(nv2[:, jt:jt + 1], psA[:, jt, :],
                                        axis=mybir.AxisListType.X, op=ALU.min)
            for jt in range(JT):
                sl = slice(jt * JW, (jt + 1) * JW)
                nc.tensor.matmul(psB[:, jt, :], w2b[:], etb[:, sl],
                                 start=True, stop=False)          # -2 M1
                nc.tensor.matmul(psB[:, jt, :], u2tb[:], eetb[:, sl],
                                 start=False, stop=False)         # M2
                nc.tensor.matmul(psB[:, jt, :], w2ee[:], ones_b[:, :JW],
                                 start=False, stop=True)          # + a_i
                nc.scalar.activation(s_sb[:, sl], psB[:, jt, :], AF.Relu)
                mp = work.tile([128, JW], BF16, tag="mp")
                nc.vector.scalar_tensor_tensor(mp[:], psA[:, jt, :],
                                               pr2[:, jt:jt + 1],
                                               s_sb[:, sl],
                                               op0=ALU.is_equal, op1=ALU.mult)
                scr_p = work.tile([128, JW], BF16, tag="scr_p")
                sh = shalf0 if jt == 0 else shalf1
                nc.scalar.activation(scr_p[:], mp[:], AF.Copy,
                                     accum_out=sh[:, 0:1])
                mn = work.tile([128, JW], BF16, tag="mn")
                nc.vector.scalar_tensor_tensor(mn[:], psA[:, jt, :],
                                               nv2[:, jt:jt + 1],
                                               s_sb[:, sl],
                                               op0=ALU.is_equal, op1=ALU.mult)
                scr_n = work.tile([128, JW], BF16, tag="scr_n")
                nc.scalar.activation(scr_n[:], mn[:], AF.Copy,
                                     accum_out=sh[:, 1:2])

            # ---------------- mining merge --------------------
            pos_raw = singles.tile([128, 1], F32)   # V + d2_pos
            nc.vector.tensor_reduce(pos_raw[:], pr2[:],
                                    axis=mybir.AxisListType.X, op=ALU.max)
            pv = singles.tile([128, 2], F32)        # [:,0]=d2_pos [:,1]=d2_neg
            nc.vector.tensor_reduce(pv[:, 1:2], nv2[:],
                                    axis=mybir.AxisListType.X, op=ALU.min)
            nc.vector.tensor_scalar(pv[:, 0:1], pos_raw[:], -SAME_V, None,
                                    op0=ALU.add)

            s_sel = singles.tile([128, 2], F32)     # S at argmax / argmin
            wsel = work.tile([128, 2], F32)
            nc.vector.tensor_tensor(wsel[:, 0:1], pr2[:, 0:1], pr2[:, 1:2],
                                    op=ALU.is_ge)   # half0 holds global max?
            nc.vector.tensor_tensor(wsel[:, 1:2], nv2[:, 0:1], nv2[:, 1:2],
                                    op=ALU.is_le)   # half0 holds global min?
            dsel = work.tile([128, 2], F32)
            nc.vector.tensor_tensor(dsel[:], shalf0[:], shalf1[:],
                                    op=ALU.subtract)
            nc.vector.tensor_tensor(dsel[:], dsel[:], wsel[:], op=ALU.mult)
            nc.vector.tensor_tensor(s_sel[:], dsel[:], shalf1[:], op=ALU.add)

            # ---------------- per-anchor tail ([128,*] small ops) --------
            t_pool = work
            # valid = hardest negative exists (d2_neg < 1e4)
            nc.vector.tensor_scalar(stats[:, 1:2], pv[:, 1:2], 1.0e4, None,
                                    op0=ALU.is_lt)
            # distance half of the tail only needs pv -> runs during the
            # eq-match phase, ahead of s_sel.
            pq = t_pool.tile([128, 2], F32)         # guarded d2 (for recip)
            nc.vector.tensor_scalar(pq[:], pv[:], 1.0e-6, None, op0=ALU.max)
            pq100 = t_pool.tile([128, 2], F32)      # 100 * guarded d2
            nc.vector.tensor_scalar(pq100[:], pv[:], 1.0e-6, 100.0,
                                    op0=ALU.max, op1=ALU.mult)
            rcp = t_pool.tile([128, 2], F32)
            nc.vector.reciprocal(rcp[:], pq[:])
            lpq = t_pool.tile([128, 2], F32)
            nc.scalar.activation(lpq[:], pq100[:], AF.Ln)
            d10 = t_pool.tile([128, 2], F32)        # 10*d_pos, 10*d_neg
            nc.scalar.activation(d10[:], lpq[:], AF.Exp, scale=0.5)
            pre = t_pool.tile([128, 1], F32)        # 10*(d_pos - d_neg)
            nc.vector.tensor_tensor(pre[:], d10[:, 0:1], d10[:, 1:2],
                                    op=ALU.subtract)
            # sigma half needs s_sel (the serial end of mining)
            u2_pn = t_pool.tile([128, 2], F32)
            nc.vector.tensor_tensor(u2_pn[:], s_sel[:], rcp[:], op=ALU.mult)
            u2sum = t_pool.tile([128, 1], F32)
            nc.vector.tensor_reduce(u2sum[:], u2_pn[:],
                                    axis=mybir.AxisListType.X, op=ALU.add)
            lg = t_pool.tile([128, 1], F32)
            nc.scalar.activation(lg[:], u2sum[:], AF.Ln,
                                 bias=b_sig[:], scale=1.0)
            sig = t_pool.tile([128, 1], F32)
            nc.scalar.activation(sig[:], lg[:], AF.Exp, scale=0.5)
            raw = t_pool.tile([128, 1], F32)        # pre + 3*sigma
            nc.vector.scalar_tensor_tensor(raw[:], sig[:], 3.0, pre[:],
                                           op0=ALU.mult, op1=ALU.add)
            # softplus(x) = relu(x) + ln(1 + exp(-|x|)), x = raw + 3
            ax = t_pool.tile([128, 1], F32)
            nc.scalar.activation(ax[:], raw[:], AF.Abs,
                                 bias=b_three[:], scale=1.0)
            en = t_pool.tile([128, 1], F32)
            nc.scalar.activation(en[:], ax[:], AF.Exp, scale=-1.0)
            l1p = t_pool.tile([128, 1], F32)
            nc.scalar.activation(l1p[:], en[:], AF.Ln, bias=1.0, scale=1.0)
            rl = t_pool.tile([128, 1], F32)
            nc.vector.tensor_scalar(rl[:], raw[:], 3.0, 0.0,
                                    op0=ALU.add, op1=ALU.max)
            pt10 = t_pool.tile([128, 1], F32)       # softplus(10*raw_ref)
            nc.vector.tensor_tensor(pt10[:], rl[:], l1p[:], op=ALU.add)
            nc.vector.tensor_tensor(stats[:, 0:1], pt10[:], stats[:, 1:2],
                                    op=ALU.mult)
            nc.gpsimd.memset(stats[:, 3:4], 0.0)

            # ---------------- final partition reduction -----------------
            ps_out = pmain.tile([1, 4], F32)
            nc.tensor.matmul(ps_out[:], ones_col[:], stats[:],
                             start=True, stop=True)
            out_sb = singles.tile([1, 4], F32)
            nc.vector.tensor_copy(out_sb[:], ps_out[:])
            nc.sync.dma_start(out[:, :], out_sb[:])

    nc.compile()
    return nc


_NC = None


def _get_nc():
    global _NC
    if _NC is None:
        _NC = build_nc()
    return _NC


def build_in_maps(embeddings, uncertainties, labels):
    emb = np.asarray(embeddings, dtype=np.float32)
    unc = np.asarray(uncertainties, dtype=np.float32)
    lab = np.asarray(labels).reshape(B).astype(np.int64)
    etf = np.ascontiguousarray(emb.T)                  # [D, B]
    etb16 = np.ascontiguousarray(etf.astype(NP_BF16))  # bf16 E^T for PE
    utf = np.ascontiguousarray(unc.T)                  # [D, B]
    onehot = np.zeros((NUM_CLASSES, B), np.float32)
    onehot[lab, np.arange(B)] = 1.0
    ohall = np.ascontiguousarray(onehot.astype(NP_BF16))
    ohv = (SAME_V * onehot).astype(NP_BF16)
    in_maps = []
    for c in range(N_CORES):
        r0, r1 = c * SH, (c + 1) * SH
        in_maps.append({
            "etb": etb16,
            "aux": np.ascontiguousarray(
                np.concatenate([etf[:, r0:r1], utf[:, r0:r1]], axis=1)),
            "ohx": np.ascontiguousarray(
                np.concatenate([ohv[:, r0:r1], ohall], axis=1)),
        })
    return in_maps


def finalize(results):
    stats = np.stack([np.asarray(results[c]["out"]).reshape(4)
                      for c in range(N_CORES)])
    tot = stats.sum(axis=0)
    main = (tot[0] / 10.0) / max(tot[1], 1.0)
    reg = tot[2] / (B * D)
    return np.float32(main + 0.05 * reg)


def kernel(embeddings, uncertainties, labels):
    nc = _get_nc()
    in_maps = build_in_maps(embeddings, uncertainties, labels)
    res = run_bass_kernel_spmd(nc, in_maps, core_ids=list(range(N_CORES)))
    return finalize(res.results)


# revision 30
# speedup vs baseline: 1.0093x; 1.0007x over previous
"""Bayesian triplet loss on 8 Trainium2 NeuronCores (Bass/Tile, SPMD).

Reference semantics:
  u   = clip(uncertainties, 1e-6, 1.0)
  d2[i,j] = ||e_i - e_j||^2            (Gram trick: n_i + n_j - 2 e_i.e_j)
  S[i,j]  = sum_k (e_ik - e_jk)^2 u_ik^2 = a_i - 2*M1[i,j] + M2[i,j]
            (a_i = sum u2*e^2, M1 = (u2*e)E^T, M2 = u2 (E*E)^T)
  mining: hardest positive (max d2 same-label), hardest negative
          (min d2 diff-label).
  per_triplet = softplus(10*(d_pos - d_neg + 0.3*(1+sigma)))/10,
          sigma = sqrt(S_pos/d2_pos + S_neg/d2_neg + 3e-8)
  loss = sum(valid*per_triplet)/max(sum(valid),1) + 0.05*mean(u)

Implementation notes:
  * Inputs are shipped host-transposed (contraction dim d on partitions)
    so every DMA lands with contiguous rows and the TensorEngine needs
    no on-device transposes.  E^T is also packed to bf16 on the host --
    identical values to the on-device cast it replaces, at half the DMA
    bytes.  All arithmetic (distances, masks, mining, loss) runs
    on-device; the host only does layout prep and the final 8-way
    partial-sum combine.
  * A short burst of dummy matmuls on constant data runs while the
    input DMAs land so PE_HAM releases the 1.2 GHz cold-clock throttle
    before the real matmuls issue.
  * Label masks are folded into the pairwise PSUM via a one-hot matmul:
    PSUM_A = d2 + V*same(i,j), V=65536 (all d2 < 500 here).  One
    free-dim max mines the hardest positive (same-label entries
    dominate), one min mines the hardest negative.  d2_pos is recovered
    exactly as max - V (both live in the 2^16 binade).
  * The diagonal needs no explicit mask: d2_ii ~ 0 can never be the
    same-label max, and singleton-label anchors have ~0 probability.
  * S at the argmax/argmin is recovered by exact float equality against
    the PSUM values, multiplied by S and accumulated on the Scalar
    engine -- no argmax/gather instruction needed.
  * n_i, n_j and a_i are all injected into PSUM with all-ones matmuls
    (one extra pass each), so no partition-direction reductions exist
    anywhere except the final [128,4] -> [1,4] ones-matmul.
  * Sharding: anchors (batch rows) split 8 ways; embeddings replicated
    per core, so no collectives.  Each core emits [1,4] partial sums;
    the host combines them (the usual data-parallel loss gather).
"""

import sys

if "/opt/trn_rl_repo" not in sys.path:
    sys.path.insert(0, "/opt/trn_rl_repo")

import numpy as np

import concourse.bacc as bacc
import concourse.mybir as mybir
from concourse import tile
from concourse.bass_utils import run_bass_kernel_spmd

# Force every activation into the one table that contains all functions
# this kernel uses (ln, exp, abs, relu, square, copy, identity).  The
# default first-match placement alternates natural_log <-> exp_and_others
# tables, costing a 1.3us ACT_TABLE_LOAD per transition.  Set ids must
# keep their act_info.json positions, so empty the other sets instead of
# reordering.
_ORIG_GAT = bacc.get_activation_tables


def _gat_single_set(arch):
    tabs = _ORIG_GAT(arch)
    keep = "natural_log_exp_and_others"
    if keep in tabs:
        return {n: (f if n == keep else set()) for n, f in tabs.items()}
    return tabs


bacc.get_activation_tables = _gat_single_set

B, D = 1024, 128
NUM_CLASSES = 64
N_CORES = 8
SH = B // N_CORES  # 128 anchor rows per core
JT = 2             # two 512-wide column tiles
JW = B // JT

F32 = mybir.dt.float32
BF16 = mybir.dt.bfloat16
NP_BF16 = mybir.dt.np(BF16)

SAME_V = 65536.0   # same-label offset; exact in bf16/f32
ALU = mybir.AluOpType
AF = mybir.ActivationFunctionType


def build_nc():
    nc = bacc.Bacc("TRN2", target_bir_lowering=False, debug=False,
                   num_devices=N_CORES)

    etb_in = nc.dram_tensor("etb", [D, B], BF16, kind="ExternalInput")
    aux = nc.dram_tensor("aux", [D, 2 * SH], F32, kind="ExternalInput")
    ohx = nc.dram_tensor("ohx", [NUM_CLASSES, SH + B], BF16,
                         kind="ExternalInput")
    out = nc.dram_tensor("out", [1, 4], F32, kind="ExternalOutput")

    with tile.TileContext(nc) as tc:
        with (
            tc.tile_pool(name="singles", bufs=1) as singles,
            tc.tile_pool(name="work", bufs=1) as work,
            tc.tile_pool(name="pmain", bufs=1, space="PSUM") as pmain,
        ):
            # ---------------- loads first (3 DMAs from 3 engines so the
            # trigger/SWDGE prep overlaps; everything else queues behind) --
            # ones_b first: it gates the PE warm-up stream
            ones_b = singles.tile([128, JW], BF16)
            nc.vector.memset(ones_b[:], 1.0)

            aux_sb = work.tile([D, 2 * SH], F32)
            nc.sync.dma_start(aux_sb[:], aux[:, :])
            etanc_sb = aux_sb[:, 0:SH]
            utanc_sb = aux_sb[:, SH:2 * SH]
            etb = work.tile([D, B], BF16)        # E^T bf16
            nc.sync.dma_start(etb[:, :JW], etb_in[:, :JW])
            nc.sync.dma_start(etb[:, JW:], etb_in[:, JW:])
            ohx_sb = work.tile([NUM_CLASSES, SH + B], BF16)
            nc.gpsimd.dma_start(ohx_sb[:], ohx[:, :])
            oha = ohx_sb[:, 0:SH]
            ohl = ohx_sb[:, SH:SH + B]

            # ---------------- remaining constants -----------------
            ones_col = singles.tile([128, 1], F32)
            nc.gpsimd.memset(ones_col[:], 1.0)
            b_sig = singles.tile([128, 1], F32)
            nc.gpsimd.memset(b_sig[:], 3.0e-8)
            b_three = singles.tile([128, 1], F32)
            nc.gpsimd.memset(b_three[:], 3.0)

            # ---------------- stats tile (written piecemeal) ------------
            stats = singles.tile([128, 4], F32)

            # PE warm-up: ~3.5us of dummy matmuls on constant data while
            # the input DMAs land, so PE_HAM releases the clock throttle
            # (1.2 -> 2.4 GHz) before the real matmuls issue.
            with tc.tile_pool(name="pwarm", bufs=1, space="PSUM") as pwarm:
                psW = pwarm.tile([128, JW], F32)
                for _ in range(9):
                    nc.tensor.matmul(psW[:], ones_b[:, :128], ones_b[:],
                                     start=True, stop=True)

            # ---------------- prep: anchor chain first (critical path) ---
            emtb = work.tile([D, SH], BF16)      # anchor E^T bf16
            nc.vector.tensor_copy(emtb[:], etanc_sb)
            nemtb = work.tile([D, SH], BF16)     # -2 * anchor E^T
            nc.scalar.mul(nemtb[:], emtb[:], -2.0)
            eetmb = work.tile([D, SH], BF16)     # anchor (E^T)^2
            nc.scalar.square(eetmb[:], emtb[:])

            # u clip (+ total-sum for the regularizer), square
            ucl = work.tile([D, SH], F32)
            nc.vector.tensor_scalar(ucl[:], utanc_sb, 1.0e-6, 1.0,
                                    op0=ALU.max, op1=ALU.min)
            u2t32 = work.tile([D, SH], F32)
            nc.scalar.square(u2t32[:], ucl[:])
            u2tb = work.tile([D, SH], BF16)
            nc.vector.tensor_copy(u2tb[:], u2t32[:])
            w2b = work.tile([D, SH], BF16)       # -2 * u2^T * E^T
            nc.vector.scalar_tensor_tensor(w2b[:], u2tb[:], -2.0, emtb[:],
                                           op0=ALU.mult, op1=ALU.mult)
            w2ee = work.tile([D, SH], BF16)      # u2^T * (E^T)^2  (a_i lhsT)
            nc.vector.tensor_tensor(w2ee[:], u2tb[:], eetmb[:], op=ALU.mult)
            nc.vector.tensor_reduce(stats[:, 2:3], ucl[:],
                                    axis=mybir.AxisListType.X, op=ALU.add)

            # bulk E^T squares
            eetb = work.tile([D, B], BF16)       # (E^T)^2
            nc.scalar.square(eetb[:, :JW], etb[:, :JW])
            nc.scalar.square(eetb[:, JW:], etb[:, JW:])

            # ---------------- main matmuls -----------------
            # PSUM_A = d2 + V*same = -2G + n_i + n_j + V*same01
            # PSUM_B = S = M2 - 2*M1 + a_i
            psA = pmain.tile([128, JT, JW], F32)
            psB = pmain.tile([128, JT, JW], F32)
            s_sb = work.tile([128, B], BF16)
            pr2 = singles.tile([128, 2], F32)    # per-half max of psA
            nv2 = singles.tile([128, 2], F32)    # per-half min of psA
            shalf0 = singles.tile([128, 2], F32)  # half0 [S_pos, S_neg]
            shalf1 = singles.tile([128, 2], F32)  # half1 [S_pos, S_neg]
            for jt in range(JT):
                sl = slice(jt * JW, (jt + 1) * JW)
                nc.tensor.matmul(psA[:, jt, :], oha, ohl[:, sl.start:sl.stop],
                                 start=True, stop=False)          # + V*same
                nc.tensor.matmul(psA[:, jt, :], nemtb[:], etb[:, sl],
                                 start=False, stop=False)         # -2 G
                nc.tensor.matmul(psA[:, jt, :], eetmb[:], ones_b[:, :JW],
                                 start=False, stop=False)         # + n_i
                nc.tensor.matmul(psA[:, jt, :], ones_b[:, :128], eetb[:, sl],
                                 start=False, stop=True)          # + n_j
                # mining as soon as this psA half is complete
                nc.vector.tensor_reduce(pr2[:, jt:jt + 1], psA[:, jt, :],
                                        axis=mybir.AxisListType.X, op=ALU.max)
                nc.vector.tensor_reduce(nv2[:, jt:jt + 1], psA[:, jt, :],
                                        axis=mybir.AxisListType.X, op=ALU.min)
            for jt in range(JT):
                sl = slice(jt * JW, (jt + 1) * JW)
                nc.tensor.matmul(psB[:, jt, :], w2b[:], etb[:, sl],
                                 start=True, stop=False)          # -2 M1
                nc.tensor.matmul(psB[:, jt, :], u2tb[:], eetb[:, sl],
                                 start=False, stop=False)         # M2
                nc.tensor.matmul(psB[:, jt, :], w2ee[:], ones_b[:, :JW],
                                 start=False, stop=True)          # + a_i
                nc.scalar.activation(s_sb[:, sl], psB[:, jt, :], AF.Relu)
                mp = work.tile([128, JW], BF16, tag="mp")
                nc.vector.scalar_tensor_tensor(mp[:], psA[:, jt, :],
                                               pr2[:, jt:jt + 1],
                                               s_sb[:, sl],
                                               op0=ALU.is_equal, op1=ALU.mult)
                scr_p = work.tile([128, JW], BF16, tag="scr_p")
                sh = shalf0 if jt == 0 else shalf1
                nc.scalar.activation(scr_p[:], mp[:], AF.Copy,
                                     accum_out=sh[:, 0:1])
                mn = work.tile([128, JW], BF16, tag="mn")
                nc.vector.scalar_tensor_tensor(mn[:], psA[:, jt, :],
                                               nv2[:, jt:jt + 1],
                                               s_sb[:, sl],
                                               op0=ALU.is_equal, op1=ALU.mult)
                if jt == 0:
                    scr_n = work.tile([128, JW], BF16, tag="scr_n")
                    nc.scalar.activation(scr_n[:], mn[:], AF.Copy,
                                         accum_out=sh[:, 1:2])
                else:
                    # last half: DVE reduce avoids queuing behind the ACT
                    # accumulate at the end of the chain
                    nc.vector.tensor_reduce(sh[:, 1:2], mn[:],
                                            axis=mybir.AxisListType.X,
                                            op=ALU.add)

            # ---------------- mining merge --------------------
            pos_raw = singles.tile([128, 1], F32)   # V + d2_pos
            nc.vector.tensor_reduce(pos_raw[:], pr2[:],
                                    axis=mybir.AxisListType.X, op=ALU.max)
            pv = singles.tile([128, 2], F32)        # [:,0]=d2_pos [:,1]=d2_neg
            nc.vector.tensor_reduce(pv[:, 1:2], nv2[:],
                                    axis=mybir.AxisListType.X, op=ALU.min)
            nc.vector.tensor_scalar(pv[:, 0:1], pos_raw[:], -SAME_V, None,
                                    op0=ALU.add)

            s_sel = singles.tile([128, 2], F32)     # S at argmax / argmin
            wsel = work.tile([128, 2], F32)
            nc.vector.tensor_tensor(wsel[:, 0:1], pr2[:, 0:1], pr2[:, 1:2],
                                    op=ALU.is_ge)   # half0 holds global max?
            nc.vector.tensor_tensor(wsel[:, 1:2], nv2[:, 0:1], nv2[:, 1:2],
                                    op=ALU.is_le)   # half0 holds global min?
            dsel = work.tile([128, 2], F32)
            nc.vector.tensor_tensor(dsel[:], shalf0[:], shalf1[:],
                                    op=ALU.subtract)
            nc.vector.tensor_tensor(dsel[:], dsel[:], wsel[:], op=ALU.mult)
            nc.vector.tensor_tensor(s_sel[:], dsel[:], shalf1[:], op=ALU.add)

            # ---------------- per-anchor tail ([128,*] small ops) --------
            t_pool = work
            # valid = hardest negative exists (d2_neg < 1e4)
            nc.vector.tensor_scalar(stats[:, 1:2], pv[:, 1:2], 1.0e4, None,
                                    op0=ALU.is_lt)
            # distance half of the tail only needs pv -> runs during the
            # eq-match phase, ahead of s_sel.
            pq = t_pool.tile([128, 2], F32)         # guarded d2 (for recip)
            nc.vector.tensor_scalar(pq[:], pv[:], 1.0e-6, None, op0=ALU.max)
            pq100 = t_pool.tile([128, 2], F32)      # 100 * guarded d2
            nc.vector.tensor_scalar(pq100[:], pv[:], 1.0e-6, 100.0,
                                    op0=ALU.max, op1=ALU.mult)
            rcp = t_pool.tile([128, 2], F32)
            nc.vector.reciprocal(rcp[:], pq[:])
            lpq = t_pool.tile([128, 2], F32)
            nc.scalar.activation(lpq[:], pq100[:], AF.Ln)
            d10 = t_pool.tile([128, 2], F32)        # 10*d_pos, 10*d_neg
            nc.scalar.activation(d10[:], lpq[:], AF.Exp, scale=0.5)
            pre = t_pool.tile([128, 1], F32)        # 10*(d_pos - d_neg)
            nc.vector.tensor_tensor(pre[:], d10[:, 0:1], d10[:, 1:2],
                                    op=ALU.subtract)
            # sigma half needs s_sel (the serial end of mining)
            u2_pn = t_pool.tile([128, 2], F32)
            nc.vector.tensor_tensor(u2_pn[:], s_sel[:], rcp[:], op=ALU.mult)
            u2sum = t_pool.tile([128, 1], F32)
            nc.vector.tensor_reduce(u2sum[:], u2_pn[:],
                                    axis=mybir.AxisListType.X, op=ALU.add)
            lg = t_pool.tile([128, 1], F32)
            nc.scalar.activation(lg[:], u2sum[:], AF.Ln,
                                 bias=b_sig[:], scale=1.0)
            sig = t_pool.tile([128, 1], F32)
            nc.scalar.activation(sig[:], lg[:], AF.Exp, scale=0.5)
            raw = t_pool.tile([128, 1], F32)        # pre + 3*sigma
            nc.vector.scalar_tensor_tensor(raw[:], sig[:], 3.0, pre[:],
                                           op0=ALU.mult, op1=ALU.add)
            # softplus(x) = relu(x) + ln(1 + exp(-|x|)), x = raw + 3
            ax = t_pool.tile([128, 1], F32)
            nc.scalar.activation(ax[:], raw[:], AF.Abs,
                                 bias=b_three[:], scale=1.0)
            en = t_pool.tile([128, 1], F32)
            nc.scalar.activation(en[:], ax[:], AF.Exp, scale=-1.0)
            l1p = t_pool.tile([128, 1], F32)
            nc.scalar.activation(l1p[:], en[:], AF.Ln, bias=1.0, scale=1.0)
            rl = t_pool.tile([128, 1], F32)
            nc.vector.tensor_scalar(rl[:], raw[:], 3.0, 0.0,
                                    op0=ALU.add, op1=ALU.max)
            pt10 = t_pool.tile([128, 1], F32)       # softplus(10*raw_ref)
            nc.vector.tensor_tensor(pt10[:], rl[:], l1p[:], op=ALU.add)
            nc.vector.tensor_tensor(stats[:, 0:1], pt10[:], stats[:, 1:2],
                                    op=ALU.mult)
            nc.gpsimd.memset(stats[:, 3:4], 0.0)

            # ---------------- final partition reduction -----------------
            ps_out = pmain.tile([1, 4], F32)
            nc.tensor.matmul(ps_out[:], ones_col[:], stats[:],
                             start=True, stop=True)
            out_sb = singles.tile([1, 4], F32)
            nc.vector.tensor_copy(out_sb[:], ps_out[:])
            nc.sync.dma_start(out[:, :], out_sb[:])

    nc.compile()
    return nc


_NC = None


def _get_nc():
    global _NC
    if _NC is None:
        _NC = build_nc()
    return _NC


def build_in_maps(embeddings, uncertainties, labels):
    emb = np.asarray(embeddings, dtype=np.float32)
    unc = np.asarray(uncertainties, dtype=np.float32)
    lab = np.asarray(labels).reshape(B).astype(np.int64)
    etf = np.ascontiguousarray(emb.T)                  # [D, B]
    etb16 = np.ascontiguousarray(etf.astype(NP_BF16))  # bf16 E^T for PE
    utf = np.ascontiguousarray(unc.T)                  # [D, B]
    onehot = np.zeros((NUM_CLASSES, B), np.float32)
    onehot[lab, np.arange(B)] = 1.0
    ohall = np.ascontiguousarray(onehot.astype(NP_BF16))
    ohv = (SAME_V * onehot).astype(NP_BF16)
    in_maps = []
    for c in range(N_CORES):
        r0, r1 = c * SH, (c + 1) * SH
        in_maps.append({
            "etb": etb16,
            "aux": np.ascontiguousarray(
                np.concatenate([etf[:, r0:r1], utf[:, r0:r1]], axis=1)),
            "ohx": np.ascontiguousarray(
                np.concatenate([ohv[:, r0:r1], ohall], axis=1)),
        })
    return in_maps


def finalize(results):
    stats = np.stack([np.asarray(results[c]["out"]).reshape(4)
                      for c in range(N_CORES)])
    tot = stats.sum(axis=0)
    main = (tot[0] / 10.0) / max(tot[1], 1.0)
    reg = tot[2] / (B * D)
    return np.float32(main + 0.05 * reg)


def kernel(embeddings, uncertainties, labels):
    nc = _get_nc()
    in_maps = build_in_maps(embeddings, uncertainties, labels)
    res = run_bass_kernel_spmd(nc, in_maps, core_ids=list(range(N_CORES)))
    return finalize(res.results)
